# revision 6
# baseline (speedup 1.0000x reference)
"""Trainium2 Bass kernel for ClothesBasedAdversarialLossWithMemoryBank.

Strategy (C-sharded over 8 cores; clothes axis split 50000 -> 8 x 6250):
  Replicated per core (cheap, B-space):
    eq[b,b'] = (t_b == t_b') via PE-transposed target broadcast + is_equal
    group-sum = eq @ inputs (PE), normalized -> mpn rows  (normalize(sum)==normalize(mean))
    inn = l2-normalized inputs; s_id = SCALE * rowdot(inn, mpn)
  Per-core shard:
    base_n = l2-normalized feature_memory rows (bf16) -> DRAM staging (2 column halves)
    indirect-DMA scatter of mpn rows at (t - c0) with OOB indices skipped
    xbar-transpose staging -> mem_nT [256, 6250] bf16 (matmul rhs layout)
    sims s = inn @ mem_nT (PE bf16, fp32 PSUM), streamed over [128b x 1024c] tiles:
      s_bf  = ACT Copy(s*16) PSUM->SBUF bf16, pos_bf = ACT Copy(pos)+accum P
      s_msk = DVE STT (pos_bf * -100 + s_bf)    # exp(s-100) ~ 0 kills positives
      ACT Exp(s_msk) + accum -> S  = sum_c e*(1-pos)   directly, no cancellation
      w = DVE STT (pos_bf * s_bf); ACT Copy(w)+accum -> W = sum_c pos*s
  AllReduce [128,24] partials (S, W, P per row), then replicated finalize:
    lnS; u = s_id - lnS; z = e^u; lp = ln(1+z)
    L_b = 0.9*(lp - u) + 0.1*(P*lnS - W + lp)/P ; loss = mean_b L_b
  (exact up to sum_{non-identity pos} [log1p(z)-z] ~ 1e-6 relative; validated.)
"""
import numpy as np

from concourse import bass, bacc, tile, mybir
from concourse.bass_utils import run_bass_kernel_spmd
from concourse.masks import make_identity

B = 1024
C = 50000
D = 256
NCORES = 8
SH = C // NCORES          # 6250
SCALE = 16.0
NB = B // 128             # 8 b-chunks
NCT = (SH + 127) // 128   # 49 c-tiles for memory normalize
CSUB = 1024               # c-subtile width in main loop
NCS = (SH + CSUB - 1) // CSUB   # 7 subtiles (6x1024 + 106)
SH_PAD = 6256             # xbar transpose needs rows %% 16 == 0

f32 = mybir.dt.float32
bf16 = mybir.dt.bfloat16
i32 = mybir.dt.int32

_CACHED_NC = None
_LAST_RESULTS = None


def build_nc():
    nc = bacc.Bacc("TRN2", target_bir_lowering=False, debug=False,
                   num_devices=NCORES)
    inputs_d = nc.dram_tensor("inputs", [B, D], f32, kind="ExternalInput")
    fm_d = nc.dram_tensor("fm", [SH, D], f32, kind="ExternalInput")
    pos_d = nc.dram_tensor("pos", [B, SH], f32, kind="ExternalInput")
    teq_d = nc.dram_tensor("t_eq", [B, 1], i32, kind="ExternalInput")
    tsc_d = nc.dram_tensor("t_scat", [B, 1], i32, kind="ExternalInput")
    loss_d = nc.dram_tensor("loss", [1, 1], f32, kind="ExternalOutput")
    stag = [nc.dram_tensor(f"stag{h}", [SH_PAD, 128], bf16) for h in range(2)]

    with tile.TileContext(nc) as tc:
        with (
            tc.tile_pool(name="persist", bufs=1) as pp,
            tc.tile_pool(name="dram", bufs=1, space="DRAM") as dp,
        ):
            # ---------------- persistent SBUF ----------------
            in_nT = [pp.tile([128, B], bf16, tag=f"in_nT{h}", name=f"in_nT{h}") for h in range(2)]
            mem_nT = [pp.tile([128, SH_PAD], bf16, tag=f"mem_nT{h}", name=f"mem_nT{h}") for h in range(2)]
            in_n_all = pp.tile([128, NB * D], bf16, tag="in_n_all")
            in_raw_all = pp.tile([128, NB * D], bf16, tag="in_raw_all")
            mpn_all = pp.tile([128, NB * D], bf16, tag="mpn_all")
            sid_all = pp.tile([128, NB], f32, tag="sid_all")
            partial = pp.tile([128, 3 * NB], f32, tag="partial")
            res_all = pp.tile([128, 3 * NB], f32, tag="res_all")
            L_all = pp.tile([128, NB], f32, tag="L_all")

            # =================== PHASE A ===================
            with (
                tc.tile_pool(name="ea_sb", bufs=3) as ea,
                tc.tile_pool(name="eq_sb", bufs=1) as eqp,
                tc.tile_pool(name="ea_ps", bufs=2, space="PSUM") as eps,
            ):
                ident = eqp.tile([128, 128], f32, tag="ident")
                make_identity(nc, ident[:])

                # targets as f32 per chunk + broadcast row [128, B]
                t_f32 = eqp.tile([128, NB], f32, tag="t_f32")
                t_bcast = eqp.tile([128, B], f32, tag="t_bcast")
                for j in range(NB):
                    t_i = ea.tile([128, 1], i32, tag="t_i")
                    nc.sync.dma_start(out=t_i[:], in_=teq_d[128 * j:128 * (j + 1), :])
                    nc.vector.tensor_copy(out=t_f32[:, j:j + 1], in_=t_i[:])
                for j in range(NB):
                    tb_ps = eps.tile([128, 128], f32, tag="tb_ps")
                    nc.tensor.transpose(
                        out=tb_ps[:],
                        in_=t_f32[:, j:j + 1].to_broadcast([128, 128]),
                        identity=ident[:])
                    nc.vector.tensor_copy(
                        out=t_bcast[:, 128 * j:128 * (j + 1)], in_=tb_ps[:])

                # inputs: load, normalize, stash raw/normalized (bf16) + f32
                inf32 = eqp.tile([128, NB * D], f32, tag="inf32")
                for i in range(NB):
                    it = ea.tile([128, D], f32, tag="in_t")
                    nc.sync.dma_start(out=it[:], in_=inputs_d[128 * i:128 * (i + 1), :])
                    nc.vector.tensor_copy(
                        out=in_raw_all[:, D * i:D * (i + 1)], in_=it[:])
                    jnk = ea.tile([128, D], f32, tag="jnk_sq")
                    ssq = ea.tile([128, 1], f32, tag="ssq")
                    nc.scalar.activation(jnk[:], it[:],
                                         mybir.ActivationFunctionType.Square,
                                         accum_out=ssq[:])
                    nrm = ea.tile([128, 1], f32, tag="nrm")
                    nc.scalar.sqrt(nrm[:], ssq[:])
                    nc.vector.tensor_scalar_max(out=nrm[:], in0=nrm[:], scalar1=1e-12)
                    inv = ea.tile([128, 1], f32, tag="inv")
                    nc.vector.reciprocal(inv[:], nrm[:])
                    nc.vector.tensor_scalar_mul(
                        out=inf32[:, D * i:D * (i + 1)], in0=it[:], scalar1=inv[:, :1])
                    nc.vector.tensor_copy(
                        out=in_n_all[:, D * i:D * (i + 1)],
                        in_=inf32[:, D * i:D * (i + 1)])

                # in_nT via PE transposes (f32 -> bf16 on PSUM copy-out)
                for i in range(NB):
                    for h in range(2):
                        tp = eps.tile([128, 128], f32, tag="tp")
                        nc.tensor.transpose(
                            out=tp[:],
                            in_=inf32[:, D * i + 128 * h:D * i + 128 * (h + 1)],
                            identity=ident[:])
                        nc.vector.tensor_copy(
                            out=in_nT[h][:, 128 * i:128 * (i + 1)], in_=tp[:])

                # eq matrix (bf16) per chunk
                eq = [eqp.tile([128, B], bf16, tag=f"eq{j}", name=f"eq{j}") for j in range(NB)]
                for j in range(NB):
                    nc.vector.tensor_tensor(
                        out=eq[j][:],
                        in0=t_f32[:, j:j + 1].to_broadcast([128, B]),
                        in1=t_bcast[:],
                        op=mybir.AluOpType.is_equal)

                # group sums -> normalized mpn rows; s_id
                for i in range(NB):
                    mp_ps = eps.tile([128, D], f32, tag="mp_ps")
                    for j in range(NB):
                        nc.tensor.matmul(
                            mp_ps[:],
                            eq[j][:, 128 * i:128 * (i + 1)],
                            in_raw_all[:, D * j:D * (j + 1)],
                            start=(j == 0), stop=(j == NB - 1))
                    jnk = ea.tile([128, D], f32, tag="jnk_sq")
                    ssq = ea.tile([128, 1], f32, tag="ssq")
                    nc.scalar.activation(jnk[:], mp_ps[:],
                                         mybir.ActivationFunctionType.Square,
                                         accum_out=ssq[:])
                    nrm = ea.tile([128, 1], f32, tag="nrm")
                    nc.scalar.sqrt(nrm[:], ssq[:])
                    nc.vector.tensor_scalar_max(out=nrm[:], in0=nrm[:], scalar1=1e-12)
                    inv = ea.tile([128, 1], f32, tag="inv")
                    nc.vector.reciprocal(inv[:], nrm[:])
                    nc.vector.tensor_scalar_mul(
                        out=mpn_all[:, D * i:D * (i + 1)], in0=mp_ps[:],
                        scalar1=inv[:, :1])
                    # s_id = SCALE * rowdot(in_n, mpn)
                    pr = ea.tile([128, D], f32, tag="pr")
                    nc.vector.tensor_tensor(
                        out=pr[:], in0=in_n_all[:, D * i:D * (i + 1)],
                        in1=mpn_all[:, D * i:D * (i + 1)],
                        op=mybir.AluOpType.mult)
                    jnk2 = ea.tile([128, D], f32, tag="jnk_sq")
                    nc.scalar.activation(jnk2[:], pr[:],
                                         mybir.ActivationFunctionType.Copy,
                                         scale=SCALE,
                                         accum_out=sid_all[:, i:i + 1])

                # memory bank: normalize rows -> staging halves (bf16)
                for k in range(NCT):
                    r0 = 128 * k
                    nr = min(128, SH - r0)
                    fmt = ea.tile([128, D], f32, tag="fmt")
                    nc.sync.dma_start(out=fmt[:nr], in_=fm_d[r0:r0 + nr, :])
                    jnk = ea.tile([128, D], f32, tag="jnk_sq")
                    ssq = ea.tile([128, 1], f32, tag="ssq")
                    nc.scalar.activation(jnk[:nr], fmt[:nr],
                                         mybir.ActivationFunctionType.Square,
                                         accum_out=ssq[:nr])
                    nrm = ea.tile([128, 1], f32, tag="nrm")
                    nc.scalar.sqrt(nrm[:nr], ssq[:nr])
                    nc.vector.tensor_scalar_max(out=nrm[:nr], in0=nrm[:nr],
                                                scalar1=1e-12)
                    inv = ea.tile([128, 1], f32, tag="inv")
                    nc.vector.reciprocal(inv[:nr], nrm[:nr])
                    bn = ea.tile([128, D], bf16, tag="bn")
                    nc.vector.tensor_scalar_mul(out=bn[:nr], in0=fmt[:nr],
                                                scalar1=inv[:nr, :1])
                    for h in range(2):
                        nc.sync.dma_start(
                            out=stag[h][r0:r0 + nr, :],
                            in_=bn[:nr, 128 * h:128 * (h + 1)])

                # scatter mpn rows into staging at t - c0 (OOB skipped)
                for i in range(NB):
                    idx = ea.tile([128, 1], i32, tag="idx")
                    nc.sync.dma_start(out=idx[:],
                                      in_=tsc_d[128 * i:128 * (i + 1), :])
                    for h in range(2):
                        nc.gpsimd.indirect_dma_start(
                            out=stag[h][:],
                            out_offset=bass.IndirectOffsetOnAxis(
                                ap=idx[:, :1], axis=0),
                            in_=mpn_all[:, D * i + 128 * h:D * i + 128 * (h + 1)],
                            in_offset=None,
                            bounds_check=SH - 1, oob_is_err=False)

            # =================== PHASE B ===================
            with (
                tc.tile_pool(name="pos_sb", bufs=4) as pb,
                tc.tile_pool(name="wrk_sb", bufs=4) as wb,
                tc.tile_pool(name="acc_sb", bufs=2) as ab,
                tc.tile_pool(name="sims_ps", bufs=3, space="PSUM") as sps,
            ):
                # transpose staged memory (bf16 xbar): [SH,128] -> [128,SH]
                for h in range(2):
                    nc.sync.dma_start_transpose(out=mem_nT[h][:], in_=stag[h][:])

                for i in range(NB):
                    acc = ab.tile([128, 3 * NCS], f32, tag="acc")
                    for cs in range(NCS):
                        c0 = CSUB * cs
                        w_ = min(CSUB, SH - c0)
                        ps = sps.tile([128, CSUB], f32, tag="ps")
                        nsl = (w_ + 511) // 512
                        for n in range(nsl):
                            n0 = 512 * n
                            nw = min(512, w_ - n0)
                            for h in range(2):
                                nc.tensor.matmul(
                                    ps[:, n0:n0 + nw],
                                    in_nT[h][:, 128 * i:128 * (i + 1)],
                                    mem_nT[h][:, c0 + n0:c0 + n0 + nw],
                                    start=(h == 0), stop=(h == 1))
                        post = pb.tile([128, CSUB], f32, tag="post")
                        nc.sync.dma_start(
                            out=post[:, :w_],
                            in_=pos_d[128 * i:128 * (i + 1), c0:c0 + w_])
                        sbf = wb.tile([128, CSUB], bf16, tag="sbf")
                        nc.scalar.activation(sbf[:, :w_], ps[:, :w_],
                                             mybir.ActivationFunctionType.Copy,
                                             scale=SCALE)
                        pbf = wb.tile([128, CSUB], bf16, tag="pbf")
                        nc.scalar.activation(pbf[:, :w_], post[:, :w_],
                                             mybir.ActivationFunctionType.Copy,
                                             accum_out=acc[:, 2 * NCS + cs:2 * NCS + cs + 1])
                        smsk = wb.tile([128, CSUB], bf16, tag="smsk")
                        nc.vector.scalar_tensor_tensor(
                            out=smsk[:, :w_], in0=pbf[:, :w_], scalar=-100.0,
                            in1=sbf[:, :w_],
                            op0=mybir.AluOpType.mult, op1=mybir.AluOpType.add)
                        ejnk = wb.tile([128, CSUB], bf16, tag="ejnk")
                        nc.scalar.activation(ejnk[:, :w_], smsk[:, :w_],
                                             mybir.ActivationFunctionType.Exp,
                                             accum_out=acc[:, cs:cs + 1])
                        wt = wb.tile([128, CSUB], bf16, tag="wt")
                        nc.vector.scalar_tensor_tensor(
                            out=wt[:, :w_], in0=pbf[:, :w_], scalar=1.0,
                            in1=sbf[:, :w_],
                            op0=mybir.AluOpType.mult, op1=mybir.AluOpType.mult)
                        wjnk = wb.tile([128, CSUB], bf16, tag="wjnk")
                        nc.scalar.activation(wjnk[:, :w_], wt[:, :w_],
                                             mybir.ActivationFunctionType.Copy,
                                             accum_out=acc[:, NCS + cs:NCS + cs + 1])
                    # fold subtile partials: cols (3cs+k) -> partial[:, 3i+k]
                    for kk in range(3):
                        nc.vector.reduce_sum(
                            out=partial[:, 3 * i + kk:3 * i + kk + 1],
                            in_=acc[:, kk * NCS:(kk + 1) * NCS],
                            axis=mybir.AxisListType.X)

            # =================== PHASE C ===================
            with (
                tc.tile_pool(name="fin_sb", bufs=2) as fb,
                tc.tile_pool(name="fin_ps", bufs=1, space="PSUM") as fps,
            ):
                cc_in = dp.tile([128, 3 * NB], f32, name="cc_in")
                cc_out = dp.tile([128, 3 * NB], f32, name="cc_out")
                nc.sync.dma_start(out=cc_in[:], in_=partial[:])
                nc.gpsimd.collective_compute(
                    "AllReduce", mybir.AluOpType.add,
                    replica_groups=[list(range(NCORES))],
                    ins=[cc_in.opt()], outs=[cc_out.opt()])
                nc.sync.dma_start(out=res_all[:], in_=cc_out[:])

                for i in range(NB):
                    Scol = res_all[:, 3 * i:3 * i + 1]
                    Wcol = res_all[:, 3 * i + 1:3 * i + 2]
                    Pcol = res_all[:, 3 * i + 2:3 * i + 3]
                    lnS = fb.tile([128, 1], f32, tag="lnS")
                    nc.scalar.activation(lnS[:], Scol,
                                         mybir.ActivationFunctionType.Ln)
                    u = fb.tile([128, 1], f32, tag="u")
                    nc.vector.tensor_tensor(out=u[:], in0=sid_all[:, i:i + 1],
                                            in1=lnS[:],
                                            op=mybir.AluOpType.subtract)
                    z = fb.tile([128, 1], f32, tag="z")
                    nc.scalar.activation(z[:], u[:],
                                         mybir.ActivationFunctionType.Exp)
                    lp = fb.tile([128, 1], f32, tag="lp")
                    nc.scalar.activation(lp[:], z[:],
                                         mybir.ActivationFunctionType.Ln,
                                         bias=1.0)
                    idt = fb.tile([128, 1], f32, tag="idt")
                    nc.vector.tensor_tensor(out=idt[:], in0=lp[:], in1=u[:],
                                            op=mybir.AluOpType.subtract)
                    r1 = fb.tile([128, 1], f32, tag="r1")
                    nc.vector.tensor_tensor(out=r1[:], in0=Pcol, in1=lnS[:],
                                            op=mybir.AluOpType.mult)
                    r2 = fb.tile([128, 1], f32, tag="r2")
                    nc.vector.tensor_tensor(out=r2[:], in0=r1[:], in1=Wcol,
                                            op=mybir.AluOpType.subtract)
                    R = fb.tile([128, 1], f32, tag="R")
                    nc.vector.tensor_tensor(out=R[:], in0=r2[:], in1=lp[:],
                                            op=mybir.AluOpType.add)
                    ip = fb.tile([128, 1], f32, tag="ip")
                    nc.vector.reciprocal(ip[:], Pcol)
                    rp = fb.tile([128, 1], f32, tag="rp")
                    nc.vector.tensor_tensor(out=rp[:], in0=R[:], in1=ip[:],
                                            op=mybir.AluOpType.mult)
                    rp1 = fb.tile([128, 1], f32, tag="rp1")
                    nc.vector.tensor_scalar_mul(out=rp1[:], in0=rp[:], scalar1=0.1)
                    nc.vector.scalar_tensor_tensor(
                        out=L_all[:, i:i + 1], in0=idt[:], scalar=0.9,
                        in1=rp1[:],
                        op0=mybir.AluOpType.mult, op1=mybir.AluOpType.add)

                ones = fb.tile([128, 1], f32, tag="ones")
                nc.vector.memset(ones[:], 1.0)
                red = fps.tile([1, NB], f32, tag="red")
                nc.tensor.matmul(red[:], ones[:], L_all[:], start=True, stop=True)
                tot = fb.tile([1, 1], f32, tag="tot")
                nc.vector.reduce_sum(out=tot[:], in_=red[:],
                                     axis=mybir.AxisListType.X)
                lossv = fb.tile([1, 1], f32, tag="lossv")
                nc.vector.tensor_scalar_mul(out=lossv[:], in0=tot[:],
                                            scalar1=1.0 / B)
                nc.sync.dma_start(out=loss_d[:], in_=lossv[:])

    nc.compile()
    return nc


def kernel(inputs, feature_memory, positive_mask, targets):
    global _CACHED_NC
    inputs = np.ascontiguousarray(np.asarray(inputs), dtype=np.float32)
    fm = np.asarray(feature_memory)
    pos = np.asarray(positive_mask)
    t = np.asarray(targets).astype(np.int64)

    if _CACHED_NC is None:
        _CACHED_NC = build_nc()
    nc = _CACHED_NC

    t_eq = t.astype(np.int32)[:, None]
    in_maps = []
    for k in range(NCORES):
        c0 = k * SH
        tl = t - c0
        tsc = np.where((tl >= 0) & (tl < SH), tl, 2**30).astype(np.int32)[:, None]
        in_maps.append({
            "inputs": inputs,
            "fm": np.ascontiguousarray(fm[c0:c0 + SH], dtype=np.float32),
            "pos": np.ascontiguousarray(pos[:, c0:c0 + SH], dtype=np.float32),
            "t_eq": t_eq,
            "t_scat": tsc,
        })
    import os
    trace = bool(os.environ.get("KERNEL_TRACE"))
    res = run_bass_kernel_spmd(nc, in_maps, list(range(NCORES)), trace=trace)
    global _LAST_RESULTS
    _LAST_RESULTS = res
    return np.float32(res.results[0]["loss"][0, 0])


if __name__ == "__main__":
    rng = np.random.default_rng(0)
    inputs = rng.standard_normal((B, D)).astype(np.float32)
    fm = rng.standard_normal((C, D)).astype(np.float32)
    t = rng.integers(0, C, B).astype(np.int64)
    pos = (rng.random((B, C)) < 0.01).astype(np.float32)
    pos[np.arange(B), t] = 1.0
    out = kernel(inputs=inputs, feature_memory=fm, positive_mask=pos, targets=t)
    print("kernel loss:", out)
